# revision 23
# baseline (speedup 1.0000x reference)
"""Grok1-style MoE (T=8192, D=2048, F=4096, E=8, top_k=2) on 8 trn2 NeuronCores.

Expert-parallel: one expert per core. The router (tiny fp32 GEMM, 0.03% of
FLOPs) runs on host to decide the token->expert sharding; each core runs the
dense fused FFN  scale * (gelu_tanh(x@wg) * (x@wu)) @ wd  over the ~2048
tokens routed to its expert (bf16 matmuls, fp32 PSUM accumulate); host
scatter-adds the two expert contributions per token.

Load balance: per-core column count C = max_e(count_e) (~2100).  An
alternative "overflow subtile" rebalance (MOE_BAL=1: M main columns + a
V-wide slot bound to a possibly-foreign expert, cutting C to ~2070) measured
net-negative on hardware — the third stationary tile per k-step costs ~19ns
per chain step and the doubled weight DMA starves the down-phase prefetch —
so it is off by default.

Device layout (everything transposed so no on-device transposes are needed):
  xt  [D, C]   bf16   gathered tokens, transposed
  wg  [32, 128, 2048] bf16  = w_gate re-tiled so wg[f] is an SBUF tile
                              [k-part, f*128+j] per k-chunk (lhsT layout)
  wu  same as wg;  wgo/wuo = same retiling of the overflow expert's weights
  wd  [16, 128, 4096] bf16  = w_down re-tiled likewise (lhsT layout); wdo same
  sc  [128, C] f32    combine weight per token, broadcast over partitions
  yt  [D, C]   f32    output, transposed

Per core:  H^T[f,c] = sum_k wg[k,f] * xt[k,c]   (PSUM, accumulating mms)
           P^T[f,c] = gelu_tanh(H^T) * U^T      (ACT + DVE, bf16 to SBUF)
           Y^T[d,c] = sum_f wd[f,d] * P^T[f,c]  (PSUM, accumulating mms)
           out = Y^T * sc                        (DVE, fp32)
C is processed in column-groups so P^T stays SBUF-resident (no DRAM spill);
weights are re-read once per group.  The first group's gate phase is built as
8 quarter-K "pause chains" with quartered weight DMAs routed on the
earliest-starting queues, so the PE starts ~3us after the DMA queues open.
"""
import os
import sys

sys.path.insert(0, "/opt/trn_rl_repo")

import ml_dtypes
import numpy as np

D = 2048
F = 4096
E = 8
KD = D // 128   # 16 k-chunks for gate/up contraction
FT = F // 128   # 32 f-tiles
DT = D // 128   # 16 d-tiles
SOFTCAP = 30.0

_compiled_cache = {}


def _col_subtiles(cg, max_n=512):
    """Split a column-group of width cg into matmul free-dim subtiles
    (hardware moving-operand limit is N=512)."""
    subs = []
    off = 0
    while off < cg:
        s = min(max_n, cg - off)
        subs.append((off, s))
        off += s
    return subs


def _groups_layout(M, V):
    """Column-group structure: [(xstart, main_width, foreign_width)].

    Main columns M split as [512, rem, 1024, 1024, ...]; the V foreign
    (overflow) columns attach to the last (widest) group, where the extra
    per-k LDWEIGHTS hides under two 512-wide matmul streams.  xstart is the
    group's first column in xt/yt.
    """
    widths = [min(512, M)]
    left = M - widths[0]
    rem = left % 1024
    if rem:
        widths.append(rem)
        left -= rem
    while left:
        widths.append(1024)
        left -= 1024
    fg = len(widths) - 1
    out = []
    xs = 0
    for i, w in enumerate(widths):
        fv = V if (V and i == fg) else 0
        out.append((xs, w, fv))
        xs += w + fv
    return out


def _build(C, M, V, maxn=512):
    import concourse.bass as bass
    import concourse.tile as tile
    from concourse import bacc, mybir

    f32 = mybir.dt.float32
    bf16 = mybir.dt.bfloat16
    GELU = mybir.ActivationFunctionType.Gelu_apprx_tanh

    assert C == M + V
    nc = bacc.Bacc("TRN2", target_bir_lowering=False, debug=False, num_devices=E, num_swdge_queues=4)
    xt = nc.dram_tensor("xt", [D, C], bf16, kind="ExternalInput").ap()
    wg = nc.dram_tensor("wg", [FT, 128, D], bf16, kind="ExternalInput").ap()
    wu = nc.dram_tensor("wu", [FT, 128, D], bf16, kind="ExternalInput").ap()
    wd = nc.dram_tensor("wd", [DT, 128, F], bf16, kind="ExternalInput").ap()
    if V:
        wgo = nc.dram_tensor("wgo", [FT, 128, D], bf16, kind="ExternalInput").ap()
        wuo = nc.dram_tensor("wuo", [FT, 128, D], bf16, kind="ExternalInput").ap()
        wdo = nc.dram_tensor("wdo", [DT, 128, F], bf16, kind="ExternalInput").ap()
    sc = nc.dram_tensor("sc", [128, C], f32, kind="ExternalInput").ap()
    yt = nc.dram_tensor("yt", [D, C], f32, kind="ExternalOutput").ap()

    groups = _groups_layout(M, V)

    with tile.TileContext(nc) as tc:
        with (
            tc.tile_pool(name="xts", bufs=1) as xpool,
            tc.tile_pool(name="pt", bufs=1) as ptpool,
            tc.tile_pool(name="wgp", bufs=7) as wgpool,
            tc.tile_pool(name="wup", bufs=3) as wupool,
            tc.tile_pool(name="wdp", bufs=3) as wdpool,
            tc.tile_pool(name="gel", bufs=2) as gpool,
            tc.tile_pool(name="scp", bufs=1) as spool,
            tc.tile_pool(name="outp", bufs=2) as opool,
            tc.tile_pool(name="ps", bufs=8, space="PSUM") as psum,
        ):
            for gi, (gx, cg, fv) in enumerate(groups):
                cgt = cg + fv
                subs = _col_subtiles(cg, maxn)
                # quarter-tiles so early k-chunk matmuls wait on only 1/4 of
                # the token DMA; alternate the two HW-DGE queues
                xq = KD // 4
                xts_q = [
                    xpool.tile([128, xq, cgt], bf16, name=f"xtsq{q}", tag=f"xtsq{q}")
                    for q in range(4)
                ]
                startup = gi == 0 and cg == 512 and fv == 0 and KD % 4 == 0
                if not startup:
                    for k in range(KD):
                        dst = xts_q[k // xq][:, k % xq, :]
                        eng = nc.sync if k % 2 == 0 else nc.scalar
                        eng.dma_start(dst, xt[k * 128 : (k + 1) * 128, gx : gx + cgt])
                pt = ptpool.tile([128, FT, cgt], bf16, name="pt")

                f_start = 0
                if startup:
                    # PE warmup: the tensor engine p-state ramps
                    # 0.65->1.2->2.4GHz over ~3us of continuous work, and the
                    # first real matmul cannot start before ~14us (engine
                    # preamble + DMA queue spin-up + first transfers).  A
                    # short chain of dummy matmuls on a memset tile runs the
                    # ramp out during the otherwise-idle wait.
                    warm = wupool.tile([128, D], bf16, name="warm", tag="wut")
                    nc.vector.memset(warm[:, :640], 0.0)
                    psD = psum.tile([128, maxn], f32, name="ps", tag="ps")
                    for _ in range(16):
                        nc.tensor.matmul(
                            psD[:, :512],
                            warm[:, :128],
                            warm[:, 128:640],
                            start=True,
                            stop=True,
                        )
                    # Startup ramp: quarter the weight DMAs of the first 7
                    # f-tiles and run 7 paused PSUM chains (one bank each),
                    # accumulating k-quarters as they land: the first matmul
                    # issues after ~0.3MB of DMA instead of ~2.6MB, and the
                    # PE stays fed by 128KB weight-quarter deliveries.
                    SF = 7
                    wgt8 = [
                        wgpool.tile([128, D], bf16, name="wgt", tag="wgt")
                        for _ in range(SF)
                    ]
                    for q in range(4):
                        for f in range(SF):
                            nc.gpsimd.dma_start(
                                wgt8[f][:, q * 512 : (q + 1) * 512],
                                wg[f, :, q * 512 : (q + 1) * 512],
                            )
                    for k in range(KD):
                        dst = xts_q[k // xq][:, k % xq, :]
                        eng = nc.sync if k % 2 == 0 else nc.scalar
                        eng.dma_start(dst, xt[k * 128 : (k + 1) * 128, gx : gx + cgt])
                    psH8 = [
                        psum.tile([128, maxn], f32, name="ps", tag="ps")
                        for _ in range(SF)
                    ]
                    for q in range(4):
                        for f in range(SF):
                            for kk in range(xq):
                                k = q * xq + kk
                                nc.tensor.matmul(
                                    psH8[f][:, :cg],
                                    wgt8[f][:, k * 128 : (k + 1) * 128],
                                    xts_q[q][:, kk, :],
                                    start=(k == 0),
                                    stop=(k == KD - 1),
                                )
                    for f in range(SF):
                        gel = gpool.tile([128, cgt], f32, name="gel")
                        nc.scalar.activation(gel[:, :cg], psH8[f][:, :cg], GELU)
                        wut = wupool.tile([128, D], bf16, name="wut")
                        nc.gpsimd.dma_start(wut[:], wu[f])
                        psU = psum.tile([128, maxn], f32, name="ps", tag="ps")
                        for k in range(KD):
                            nc.tensor.matmul(
                                psU[:, :cg],
                                wut[:, k * 128 : (k + 1) * 128],
                                xts_q[k // xq][:, k % xq, :],
                                start=(k == 0),
                                stop=(k == KD - 1),
                            )
                        nc.vector.tensor_mul(pt[:, f, :cg], gel[:, :cg], psU[:, :cg])
                    f_start = SF

                for f in range(f_start, FT):
                    wgt = wgpool.tile([128, D], bf16, name="wgt")
                    nc.gpsimd.dma_start(wgt[:], wg[f])
                    if fv:
                        wgto = wgpool.tile([128, D], bf16, name="wgto", tag="wgt")
                        nc.gpsimd.dma_start(wgto[:], wgo[f])
                    wut = wupool.tile([128, D], bf16, name="wut")
                    nc.gpsimd.dma_start(wut[:], wu[f])
                    if fv:
                        wuto = wupool.tile([128, D], bf16, name="wuto", tag="wut")
                        nc.gpsimd.dma_start(wuto[:], wuo[f])

                    psH = [psum.tile([128, maxn], f32, name="ps", tag="ps") for _ in subs]
                    if fv:
                        psHF = psum.tile([128, maxn], f32, name="ps", tag="ps")
                    for k in range(KD):
                        lhs = wgt[:, k * 128 : (k + 1) * 128]
                        for ci, (off, s) in enumerate(subs):
                            nc.tensor.matmul(
                                psH[ci][:, :s],
                                lhs,
                                xts_q[k // xq][:, k % xq, off : off + s],
                                start=(k == 0),
                                stop=(k == KD - 1),
                            )
                        if fv:
                            nc.tensor.matmul(
                                psHF[:, :fv],
                                wgto[:, k * 128 : (k + 1) * 128],
                                xts_q[k // xq][:, k % xq, cg : cg + fv],
                                start=(k == 0),
                                stop=(k == KD - 1),
                            )
                    gel = gpool.tile([128, cgt], f32, name="gel")
                    for ci, (off, s) in enumerate(subs):
                        nc.scalar.activation(gel[:, off : off + s], psH[ci][:, :s], GELU)
                    if fv:
                        nc.scalar.activation(gel[:, cg : cg + fv], psHF[:, :fv], GELU)

                    psU = [psum.tile([128, maxn], f32, name="ps", tag="ps") for _ in subs]
                    if fv:
                        psUF = psum.tile([128, maxn], f32, name="ps", tag="ps")
                    for k in range(KD):
                        lhs = wut[:, k * 128 : (k + 1) * 128]
                        for ci, (off, s) in enumerate(subs):
                            nc.tensor.matmul(
                                psU[ci][:, :s],
                                lhs,
                                xts_q[k // xq][:, k % xq, off : off + s],
                                start=(k == 0),
                                stop=(k == KD - 1),
                            )
                        if fv:
                            nc.tensor.matmul(
                                psUF[:, :fv],
                                wuto[:, k * 128 : (k + 1) * 128],
                                xts_q[k // xq][:, k % xq, cg : cg + fv],
                                start=(k == 0),
                                stop=(k == KD - 1),
                            )
                    for ci, (off, s) in enumerate(subs):
                        nc.vector.tensor_mul(
                            pt[:, f, off : off + s],
                            gel[:, off : off + s],
                            psU[ci][:, :s],
                        )
                    if fv:
                        nc.vector.tensor_mul(
                            pt[:, f, cg : cg + fv],
                            gel[:, cg : cg + fv],
                            psUF[:, :fv],
                        )

                sct = spool.tile([128, cgt], f32, name="sct")
                nc.gpsimd.dma_start(sct[:], sc[:, gx : gx + cgt])
                for d in range(DT):
                    wdt = wdpool.tile([128, F], bf16, name="wdt")
                    nc.gpsimd.dma_start(wdt[:], wd[d])
                    if fv:
                        wdto = wdpool.tile([128, F], bf16, name="wdto", tag="wdt")
                        nc.gpsimd.dma_start(wdto[:], wdo[d])
                    psY = [psum.tile([128, maxn], f32, name="ps", tag="ps") for _ in subs]
                    if fv:
                        psYF = psum.tile([128, maxn], f32, name="ps", tag="ps")
                    for f in range(FT):
                        lhs = wdt[:, f * 128 : (f + 1) * 128]
                        for ci, (off, s) in enumerate(subs):
                            nc.tensor.matmul(
                                psY[ci][:, :s],
                                lhs,
                                pt[:, f, off : off + s],
                                start=(f == 0),
                                stop=(f == FT - 1),
                            )
                        if fv:
                            nc.tensor.matmul(
                                psYF[:, :fv],
                                wdto[:, f * 128 : (f + 1) * 128],
                                pt[:, f, cg : cg + fv],
                                start=(f == 0),
                                stop=(f == FT - 1),
                            )
                    if gi == len(groups) - 1 and d == DT - 1:
                        # final drain: 256-col chunks on alternating queues so
                        # the last DVE muls and output DMAs pipeline instead of
                        # serializing after the last matmul
                        nch = 0
                        for ci, (off, s) in enumerate(subs):
                            for o2 in range(0, s, 256):
                                s2 = min(256, s - o2)
                                outt = opool.tile(
                                    [128, maxn], f32, name="outt", tag="outt"
                                )
                                nc.vector.tensor_mul(
                                    outt[:, :s2],
                                    psY[ci][:, o2 : o2 + s2],
                                    sct[:, off + o2 : off + o2 + s2],
                                )
                                eng = nc.sync if nch % 2 == 0 else nc.scalar
                                eng.dma_start(
                                    yt[
                                        d * 128 : (d + 1) * 128,
                                        gx + off + o2 : gx + off + o2 + s2,
                                    ],
                                    outt[:, :s2],
                                )
                                nch += 1
                    else:
                        for ci, (off, s) in enumerate(subs):
                            outt = opool.tile([128, maxn], f32, name="outt", tag="outt")
                            nc.vector.tensor_mul(
                                outt[:, :s], psY[ci][:, :s], sct[:, off : off + s]
                            )
                            nc.sync.dma_start(
                                yt[d * 128 : (d + 1) * 128, gx + off : gx + off + s],
                                outt[:, :s],
                            )
                    if fv:
                        outt = opool.tile([128, maxn], f32, name="outt", tag="outt")
                        nc.vector.tensor_mul(
                            outt[:, :fv], psYF[:, :fv], sct[:, cg : cg + fv]
                        )
                        nc.sync.dma_start(
                            yt[d * 128 : (d + 1) * 128, gx + cg : gx + cg + fv],
                            outt[:, :fv],
                        )

    nc.compile()
    return nc


def _plan_balance(counts):
    """Pick (M, V): M main columns (own expert) + one V-wide overflow slot
    per core; the 8 slots absorb every expert's spill over M.  Returns the
    (M, V) minimizing C = M + V, or None if the plain scheme is as good."""
    cmax = int(counts.max())
    cplain = int(np.ceil(cmax / 2) * 2)
    best = None
    for Vc in range(8, 136, 4):
        lo = max(520, cmax - 8 * Vc)
        for M in range(lo + (lo % 2), cmax + 1, 2):
            slots = int(sum(int(np.ceil(max(0, int(c) - M) / Vc)) for c in counts))
            if slots <= E:
                Cc = M + Vc
                if best is None or Cc < best[0]:
                    best = (Cc, M, Vc)
                break
    if best is None or best[0] >= cplain:
        return None
    return best[1], best[2]


def _enable_ntff_tracing():
    """Register the axon NTFF profile hook (the image lacks antenv.axon_hooks,
    so trn_boot's registration silently degraded). Also stub the S3 artifact
    upload, which has no credentials in this container."""
    import types

    try:
        from antenv import axon_hooks  # noqa: F401
    except ImportError:
        import antenv

        mod = types.ModuleType("antenv.axon_hooks")
        holder = [None]
        mod.set_axon_ntff_profile_hook = lambda h: holder.__setitem__(0, h)
        mod.get_axon_ntff_profile_hook = lambda: holder[0]
        sys.modules["antenv.axon_hooks"] = mod
        antenv.axon_hooks = mod
        from trn_agent_boot.trn_boot import _ntff_profile_via_ctypes

        hook = _ntff_profile_via_ctypes("/opt/axon/libaxon_pjrt.so")
        if hook is not None:
            mod.set_axon_ntff_profile_hook(hook)
    from concourse import bass_utils as bu

    bu.upload_artifacts = lambda tmpdir: ""


def kernel(hidden_states, gate_w, w_gate, w_up, w_down, top_k):
    from concourse.bass_utils import run_bass_kernel_spmd

    x = np.ascontiguousarray(np.asarray(hidden_states, dtype=np.float32))
    gw = np.asarray(gate_w, dtype=np.float32)
    k = int(top_k)
    T = x.shape[0]

    # --- host router (matches reference: fp32 gate, tanh softcap, softmax) ---
    logits = (x @ gw).astype(np.float32)
    logits = np.tanh(logits / SOFTCAP) * SOFTCAP
    m = logits.max(axis=1, keepdims=True)
    ex = np.exp(logits - m)
    probs = (ex / ex.sum(axis=1, keepdims=True)).astype(np.float32)
    order = np.argsort(-probs, axis=1, kind="stable")[:, :k]

    tok_ids = []
    counts = np.zeros(E, np.int64)
    sel = np.zeros((T, E), bool)
    for j in range(k):
        sel[np.arange(T), order[:, j]] = True
    for e in range(E):
        ids = np.nonzero(sel[:, e])[0]
        tok_ids.append(ids)
        counts[e] = len(ids)

    # The overflow-slot rebalance (MOE_BAL=1) measures net-negative on hw:
    # the extra per-k LDWEIGHTS in the foreign subtile's chains costs ~19ns
    # per chain step and the doubled weight DMA starves the down-phase
    # prefetch, together outweighing the ~18us saved by the smaller C.
    plan = None
    if int(os.environ.get("MOE_BAL", "0")):
        plan = _plan_balance(counts)
    if plan is None:
        M, V = max(256, int(np.ceil(counts.max() / 2) * 2)), 0
    else:
        M, V = plan
    C = M + V

    # overflow slot packing: chop each expert's spill over M into <=V chunks
    slot_exp = list(range(E))
    slot_ids = [np.zeros(0, np.int64) for _ in range(E)]
    if V:
        si = 0
        for e in range(E):
            spill = tok_ids[e][M:]
            for o in range(0, len(spill), V):
                slot_exp[si] = e
                slot_ids[si] = spill[o : o + V]
                si += 1

    maxn = int(os.environ.get("MOE_MAXN", "512"))
    key = (C, M, V, maxn)
    nc = _compiled_cache.get(key)
    if nc is None:
        nc = _build(C, M, V, maxn)
        _compiled_cache[key] = nc

    # main columns occupy positions [0,512)+[512,512+rem)+... with the V
    # foreign columns spliced in at the fg group's end — mirror _build
    groups = _groups_layout(M, V)
    pos_main = []
    fstart = None
    for gx, cg, fv in groups:
        pos_main.extend(range(gx, gx + cg))
        if fv:
            fstart = gx + cg
    pos_main = np.asarray(pos_main, np.int64)

    bf = ml_dtypes.bfloat16
    retile = {}
    for e in range(E):
        wg_r = np.ascontiguousarray(
            np.asarray(w_gate[e], np.float32)
            .reshape(KD, 128, FT, 128)
            .transpose(2, 1, 0, 3)
            .reshape(FT, 128, D)
        ).astype(bf)
        wu_r = np.ascontiguousarray(
            np.asarray(w_up[e], np.float32)
            .reshape(KD, 128, FT, 128)
            .transpose(2, 1, 0, 3)
            .reshape(FT, 128, D)
        ).astype(bf)
        wd_r = np.ascontiguousarray(
            np.asarray(w_down[e], np.float32)
            .reshape(FT, 128, DT, 128)
            .transpose(2, 1, 0, 3)
            .reshape(DT, 128, F)
        ).astype(bf)
        retile[e] = (wg_r, wu_r, wd_r)

    in_maps = []
    n_main = [0] * E
    for e in range(E):
        ids = tok_ids[e][:M]
        n = len(ids)
        n_main[e] = n
        xcols = np.zeros((C, D), np.float32)
        s = np.zeros((C,), np.float32)
        xcols[pos_main[:n]] = x[ids]
        s[pos_main[:n]] = probs[ids, e]
        if V:
            fids = slot_ids[e]
            nf = len(fids)
            if nf:
                xcols[fstart : fstart + nf] = x[fids]
                s[fstart : fstart + nf] = probs[fids, slot_exp[e]]
        xt = np.ascontiguousarray(xcols.T.astype(bf))
        sc = np.broadcast_to(s[None, :], (128, C)).copy()
        wg_r, wu_r, wd_r = retile[e]
        im = {"xt": xt, "wg": wg_r, "wu": wu_r, "wd": wd_r, "sc": sc}
        if V:
            og, ou, od = retile[slot_exp[e]]
            im.update({"wgo": og, "wuo": ou, "wdo": od})
        in_maps.append(im)

    trace = bool(int(os.environ.get("MOE_TRACE", "0")))
    if trace:
        try:
            _enable_ntff_tracing()
        except Exception as exc:  # tracing is best-effort, never block results
            print(f"ntff tracing unavailable: {exc!r}")
            trace = False
    res = run_bass_kernel_spmd(nc, in_maps, list(range(E)), trace=trace)
    if trace:
        kernel.last_exec_time_ns = res.exec_time_ns
        kernel.last_trace = res.instructions_and_trace

    out = np.zeros((T, D), np.float32)
    for e in range(E):
        ytc = res.results[e]["yt"]
        n = n_main[e]
        if n:
            out[tok_ids[e][:n]] += ytc[:, pos_main[:n]].T
        if V and len(slot_ids[e]):
            nf = len(slot_ids[e])
            out[slot_ids[e]] += ytc[:, fstart : fstart + nf].T
    return out
